# revision 14
# baseline (speedup 1.0000x reference)
"""Trainium2 Bass kernel for nn_Aggregator (retrieval_knn).

Reference computation: for each of B*T*Ro*S = 524288 query points, find the
8 nearest of 512 keypoints (per batch), threshold at R=0.12, cap at 48 valid
points per ray (64 points), emit (neighbor_idx, shading_pts, neighbor_dist,
mask).

Device part (per core, SPMD over 8 cores; core = (batch, ray-half)):
  - PE matmul computes s[q,m] = 2*q.k - |k|^2  (= a2[q] - d2[q,m]) for a
    128-query subtile against C candidate keypoints.
  - DVE max/max_index extract the top-8 values (= 8 smallest d2) + indices.
  - d2 = a2 - val, clamped, sqrt -> 8 ascending distances per query.
Host part: radius mask, per-ray cumsum cap, -1/0 fills, mask construction
(cheap O(N) numpy, exactly mirroring the reference semantics).
"""

import os
import sys

import numpy as np

sys.path.insert(0, "/opt/trn_rl_repo")

from contextlib import ExitStack

import concourse.bass as bass
import concourse.tile as tile
from concourse import bacc, mybir
from concourse.bass_utils import run_bass_kernel_spmd

# Problem constants
B, T, RO, S, _D = 4, 2, 1024, 64, 3
NKP = 512
K = 8
R = 0.12
MAX_SHADING_PTS = 48

N_CORES = 8
NQ_CORE = (B * T * RO * S) // N_CORES  # 65536 queries per core

F32 = mybir.dt.float32
F32R = mybir.dt.float32r
U32 = mybir.dt.uint32


def build_program(nsub, c, nrhs, g, reps=1):
    """Build the Bass program.

    nsub: number of 128-query subtiles per core
    c:    candidate keypoints per subtile
    nrhs: number of rhs matrices (1 = shared by all subtiles, else nsub)
    g:    subtiles per output-staging group (nsub % g == 0)
    reps: repeat the whole computation (timing only)

    Outputs one merged tensor oboth [n, 16] uint32: per query row,
    cols 0:8 = f32 bits of the 8 ascending distances, cols 8:16 = the
    raw candidate indices (uint32).
    """
    assert nsub % g == 0
    n = nsub * 128
    nc = bacc.Bacc("TRN2", target_bir_lowering=False)
    lhs = nc.declare_dram_parameter("lhs", [4, n], F32, isOutput=False)
    rhs = nc.declare_dram_parameter("rhs", [nrhs, 4, c], F32, isOutput=False)
    a2d = nc.declare_dram_parameter("a2d", [128, nsub], F32, isOutput=False)
    oboth = nc.declare_dram_parameter("oboth", [n, 16], U32, isOutput=True)

    with tile.TileContext(nc) as tc, ExitStack() as ctx:
        if reps > 1:
            ctx.enter_context(tc.For_i(0, reps, 1))
        lpool = ctx.enter_context(tc.tile_pool(name="lhs", bufs=2))
        rpool = ctx.enter_context(tc.tile_pool(name="rhs", bufs=2))
        apool = ctx.enter_context(tc.tile_pool(name="a2", bufs=1))
        ppool = ctx.enter_context(tc.tile_pool(name="psum", bufs=6, space="PSUM"))
        dpool = ctx.enter_context(tc.tile_pool(name="d2", bufs=4))
        wpool = ctx.enter_context(tc.tile_pool(name="wide", bufs=2))

        a2all = apool.tile([128, nsub], F32)
        nc.sync.dma_start(out=a2all[:], in_=a2d[:, :])
        rall = None
        if nrhs == 1:
            rall = rpool.tile([4, c], F32)
            nc.sync.dma_start(out=rall[:], in_=rhs[0])

        for m in range(nsub // g):
            wv = wpool.tile([128, g * 8], F32)
            stg = wpool.tile([128, g * 16], F32)
            stg3 = stg[:].rearrange("p (g e) -> p g e", e=16)
            lc = lpool.tile([4, g * 128], F32)
            nc.sync.dma_start(
                out=lc[:], in_=lhs[:, m * g * 128 : (m + 1) * g * 128]
            )
            rc = None
            if nrhs > 1:
                rc = rpool.tile([4, g * c], F32)
                nc.sync.dma_start(
                    out=rc[:].rearrange("f (s c) -> f s c", s=g),
                    in_=rhs[m * g : (m + 1) * g].rearrange("s f c -> f s c"),
                )
            for j in range(g):
                lslice = lc[:, j * 128 : (j + 1) * 128]
                rslice = rall[:] if nrhs == 1 else rc[:, j * c : (j + 1) * c]
                psum = ppool.tile([128, c], F32)
                nc.tensor.matmul(
                    psum[:], lhsT=lslice, rhs=rslice, start=True, stop=True
                )
                d2t = dpool.tile([128, c], F32)
                nc.scalar.copy(out=d2t[:], in_=psum[:])
                nc.vector.max(wv[:, j * 8 : (j + 1) * 8], d2t[:])
                nc.vector.max_index(
                    stg3[:, j, 8:16].bitcast(U32),
                    wv[:, j * 8 : (j + 1) * 8],
                    d2t[:],
                )
            # d2 = a2 - val (clamped to >= 1e-12), dist = sqrt(d2)
            a2x = wpool.tile([128, g * 8], F32)
            a2x3 = a2x[:].rearrange("p (g e) -> p g e", e=8)
            for e in range(8):
                nc.gpsimd.tensor_copy(a2x3[:, :, e], a2all[:, m * g : (m + 1) * g])
            d2n = wpool.tile([128, g * 8], F32)
            # d2n = val - a2 = -(d2);  min with -1e-12  ==  -(max(d2, 1e-12))
            nc.gpsimd.tensor_sub(d2n[:], wv[:], a2x[:])
            nc.gpsimd.tensor_scalar_min(d2n[:], d2n[:], -1e-12)
            nc.scalar.activation(
                stg3[:, :, 0:8],
                d2n[:].rearrange("p (g e) -> p g e", e=8),
                func=mybir.ActivationFunctionType.Sqrt,
                scale=-1.0,
            )
            orows = oboth[m * g * 128 : (m + 1) * g * 128].rearrange(
                "(p g) e -> p (g e)", p=128
            )
            nc.sync.dma_start(out=orows, in_=stg[:].bitcast(U32))
    nc.compile()
    return nc


# ---------------------------------------------------------------------------
# Host-side orchestration
# ---------------------------------------------------------------------------

_PROG_CACHE = {}


def _get_program(nsub, c, nrhs, g):
    key = (nsub, c, nrhs, g)
    if key not in _PROG_CACHE:
        _PROG_CACHE[key] = build_program(nsub, c, nrhs, g)
    return _PROG_CACHE[key]


def _core_inputs_v1(q, kp):
    """Simple dense config: one rhs of all 512 keypoints, identity order.

    q:  [65536, 3] float32 queries of this core
    kp: [512, 3] float32 keypoints of this core's batch
    Returns in_map dict. Device row r maps to query  (s*128 + p)  with
    s = (r // (128*g))*g + r % g,  p = (r // g) % 128.
    """
    nsub, c, g = NQ_CORE // 128, NKP, 64
    lhs = np.empty((4, NQ_CORE), np.float32)
    lhs[:3] = q.T
    lhs[3] = 1.0
    b2 = (kp[:, 0] * kp[:, 0] + kp[:, 1] * kp[:, 1]) + kp[:, 2] * kp[:, 2]
    rhs = np.empty((1, 4, c), np.float32)
    rhs[0, :3] = 2.0 * kp.T
    rhs[0, 3] = -b2
    a2 = (q[:, 0] * q[:, 0] + q[:, 1] * q[:, 1]) + q[:, 2] * q[:, 2]
    a2d = np.ascontiguousarray(a2.reshape(nsub, 128).T)
    return {"lhs": lhs, "rhs": rhs, "a2d": a2d}


def _devrow_to_query(nsub, g):
    """origq[r] for device output row r (v1 ordering)."""
    r = np.arange(nsub * 128)
    m = r // (128 * g)
    j = r % g
    p = (r // g) % 128
    return (m * g + j) * 128 + p


def _devrow_to_slot(nsub, g):
    """slot (= s*128 + p) for device output row r."""
    r = np.arange(nsub * 128)
    m = r // (128 * g)
    j = r % g
    p = (r // g) % 128
    return (m * g + j) * 128 + p


# --- v2: spatial-cell candidate pruning ------------------------------------

D_GRID = 5
C_CAND = 80
G_V2 = 32


def _cell_candidates(kp, d, c):
    """Per-cell rhs [d^3+1, 4, c] and candidate id map [d^3+1, c].

    Cell cc covers box [i,j,l]/d..([i,j,l]+1)/d; candidates are keypoints
    within R+1e-3 of the box. Last row = dummy (all padded) for pad subtiles.
    Pad columns get [0,0,0,-1e30] so their score 2ab-b2 = -1e30 never wins.
    """
    ncell = d**3
    rhs = np.zeros((ncell + 1, 4, c), np.float32)
    rhs[:, 3, :] = -1e30
    cmap = np.zeros((ncell + 1, c), np.int32)
    kp64 = kp.astype(np.float64)
    side = 1.0 / d
    b2 = (kp[:, 0] * kp[:, 0] + kp[:, 1] * kp[:, 1]) + kp[:, 2] * kp[:, 2]
    thr = (R + 1e-3) ** 2
    for i in range(d):
        for j in range(d):
            for l in range(d):
                cc = (i * d + j) * d + l
                lo = np.array([i, j, l]) * side
                dd = np.maximum(np.maximum(lo - kp64, 0), kp64 - (lo + side))
                ids = np.nonzero((dd * dd).sum(1) < thr)[0]
                n = len(ids)
                assert n <= c, f"cell {cc}: {n} candidates > C={c}"
                rhs[cc, 0, :n] = 2.0 * kp[ids, 0]
                rhs[cc, 1, :n] = 2.0 * kp[ids, 1]
                rhs[cc, 2, :n] = 2.0 * kp[ids, 2]
                rhs[cc, 3, :n] = -b2[ids]
                cmap[cc, :n] = ids
    return rhs, cmap


def _assign_subtiles(q, d):
    """Sort queries into cells; chunk each cell into 128-query subtiles.

    Returns (perm2 [nsub_used*128] orig-query index with -1 padding,
             sub_cell [nsub_used] cell id per subtile).
    """
    nq = q.shape[0]
    cid = np.clip((q * d).astype(np.int32), 0, d - 1)
    cell = (cid[:, 0] * d + cid[:, 1]) * d + cid[:, 2]
    order = np.argsort(cell, kind="stable")
    counts = np.bincount(cell, minlength=d**3)
    perm2 = []
    sub_cell = []
    start = 0
    for cc in range(d**3):
        n = int(counts[cc])
        qs = order[start : start + n]
        start += n
        for o in range(0, n, 128):
            chunk = qs[o : o + 128]
            if len(chunk) < 128:
                chunk = np.concatenate(
                    [chunk, np.full(128 - len(chunk), -1, np.int64)]
                )
            perm2.append(chunk)
            sub_cell.append(cc)
    return np.concatenate(perm2), np.asarray(sub_cell, np.int64)


def _core_inputs_v2(q, kp, nsub, c, g):
    """Spatial config inputs + mappings for one core."""
    perm2, sub_cell = _assign_subtiles(q, D_GRID)
    nsub_used = len(sub_cell)
    assert nsub_used <= nsub, f"{nsub_used} subtiles > program NSUB={nsub}"
    perm2 = np.concatenate(
        [perm2, np.full((nsub - nsub_used) * 128, -1, np.int64)]
    )
    sub_cell = np.concatenate(
        [sub_cell, np.full(nsub - nsub_used, D_GRID**3, np.int64)]
    )

    cell_rhs, cell_cmap = _cell_candidates(kp, D_GRID, c)
    rhs = cell_rhs[sub_cell]  # [nsub, 4, c]
    cmap = cell_cmap[sub_cell]  # [nsub, c]

    qsafe = np.where(perm2 >= 0, perm2, 0)
    qc = q[qsafe]  # [nsub*128, 3] slot-ordered coords
    lhs = np.empty((4, nsub * 128), np.float32)
    lhs[:3] = qc.T
    lhs[3] = 1.0
    a2 = (qc[:, 0] * qc[:, 0] + qc[:, 1] * qc[:, 1]) + qc[:, 2] * qc[:, 2]
    a2d = np.ascontiguousarray(a2.reshape(nsub, 128).T)
    return {"lhs": lhs, "rhs": rhs, "a2d": a2d}, perm2, cmap


def kernel_v2(x, kp_pos):
    x = np.asarray(x, dtype=np.float32)
    kp_pos = np.asarray(kp_pos, dtype=np.float32)
    rays = T * RO
    c, g = C_CAND, G_V2

    xq = x.reshape(B, 2, rays // 2 * S, 3)
    preps = []
    nsub_needed = 0
    for core in range(N_CORES):
        b, half = divmod(core, 2)
        perm2, sub_cell = _assign_subtiles(xq[b, half], D_GRID)
        nsub_needed = max(nsub_needed, len(sub_cell))
    nsub = ((nsub_needed + g - 1) // g) * g

    in_maps, perms, cmaps = [], [], []
    for core in range(N_CORES):
        b, half = divmod(core, 2)
        im, perm2, cmap = _core_inputs_v2(xq[b, half], kp_pos[b], nsub, c, g)
        in_maps.append(im)
        perms.append(perm2)
        cmaps.append(cmap)

    nc = _get_program(nsub, c, nsub, g)
    res = run_bass_kernel_spmd(nc, in_maps, core_ids=list(range(N_CORES)))

    slot_of_devrow = _devrow_to_slot(nsub, g)
    devrow_of_slot = np.empty_like(slot_of_devrow)
    devrow_of_slot[slot_of_devrow] = np.arange(slot_of_devrow.size)

    vals = np.empty((B, rays * S, 8), np.float32)
    idx = np.empty((B, rays * S, 8), np.int32)
    half_n = rays // 2 * S
    s_of_slot = np.arange(nsub * 128) // 128
    for core in range(N_CORES):
        b, half = divmod(core, 2)
        ob = res.results[core]["oboth"]
        od = ob.view(np.float32)[:, :8][devrow_of_slot]  # slot order
        oi = ob.view(np.int32)[:, 8:][devrow_of_slot]
        kpid = cmaps[core][s_of_slot[:, None], oi]  # [n, 8]
        perm2 = perms[core]
        valid = perm2 >= 0
        dst = vals[b, half * half_n : (half + 1) * half_n]
        dsti = idx[b, half * half_n : (half + 1) * half_n]
        dst[perm2[valid]] = od[valid]
        dsti[perm2[valid]] = kpid[valid]

    return _postprocess(x, vals, idx)


def kernel(x, kp_pos):
    x = np.asarray(x, dtype=np.float32)
    kp_pos = np.asarray(kp_pos, dtype=np.float32)
    rays = T * RO

    nsub, c, nrhs, g = NQ_CORE // 128, NKP, 1, 64
    nc = _get_program(nsub, c, nrhs, g)

    xq = x.reshape(B, 2, rays // 2 * S, 3)
    in_maps = []
    for core in range(N_CORES):
        b, half = divmod(core, 2)
        in_maps.append(_core_inputs_v1(xq[b, half], kp_pos[b]))

    res = run_bass_kernel_spmd(nc, in_maps, core_ids=list(range(N_CORES)))

    perm = _devrow_to_query(nsub, g)
    inv = np.empty_like(perm)
    inv[perm] = np.arange(perm.size)

    vals = np.empty((B, rays * S, 8), np.float32)
    idx = np.empty((B, rays * S, 8), np.int32)
    half_n = rays // 2 * S
    for core in range(N_CORES):
        b, half = divmod(core, 2)
        ob = res.results[core]["oboth"]
        od = ob.view(np.float32)[:, :8]
        oi = ob.view(np.int32)[:, 8:]
        vals[b, half * half_n : (half + 1) * half_n] = od[inv]
        idx[b, half * half_n : (half + 1) * half_n] = oi[inv]

    return _postprocess(x, vals, idx)


def _postprocess(x, vals, idx):
    """vals: [B, rays*S, 8] ascending distances; idx: keypoint ids (0..511)."""
    rays = T * RO
    vals = vals.reshape(B, rays, S, 8)
    idx = idx.reshape(B, rays, S, 8)

    valid_nb = vals < R
    offset = (NKP * np.arange(B, dtype=np.int32)).reshape(B, 1, 1, 1)
    nb_idx = np.where(valid_nb, idx + offset, -1).astype(np.int32)

    valid_pts = valid_nb[..., 0:1]  # any() == slot 0 since ascending
    csum = np.cumsum(valid_pts.astype(np.int32), axis=-2)
    valid_pts = np.logical_and(valid_pts, csum <= MAX_SHADING_PTS)

    nb_idx = np.where(valid_pts, nb_idx, -1)
    nb_dist = np.where(np.logical_and(valid_pts, valid_nb), vals, 0.0).astype(
        np.float32
    )
    shading = np.where(valid_pts, x.reshape(B, rays, S, 3), 0.0).astype(np.float32)

    num_valid = valid_pts.sum(axis=-2, keepdims=True)
    mask = np.arange(MAX_SHADING_PTS).reshape(1, 1, -1, 1) < num_valid

    return (
        nb_idx.reshape(B, T, RO, S, K),
        shading.reshape(B, T, RO, S, 3),
        nb_dist.reshape(B, T, RO, S, K),
        mask.reshape(B, T, RO, MAX_SHADING_PTS, 1),
    )


# revision 15
# speedup vs baseline: 1.0014x; 1.0014x over previous
"""Trainium2 Bass kernel for nn_Aggregator (retrieval_knn).

Reference computation: for each of B*T*Ro*S = 524288 query points, find the
8 nearest of 512 keypoints (per batch), threshold at R=0.12, cap at 48 valid
points per ray (64 points), emit (neighbor_idx, shading_pts, neighbor_dist,
mask).

Device part (per core, SPMD over 8 cores; core = (batch, ray-half)):
  - PE matmul computes s[q,m] = 2*q.k - |k|^2  (= a2[q] - d2[q,m]) for a
    128-query subtile against C candidate keypoints.
  - DVE max/max_index extract the top-8 values (= 8 smallest d2) + indices.
  - d2 = a2 - val, clamped, sqrt -> 8 ascending distances per query.
Host part: radius mask, per-ray cumsum cap, -1/0 fills, mask construction
(cheap O(N) numpy, exactly mirroring the reference semantics).
"""

import os
import sys

import numpy as np

sys.path.insert(0, "/opt/trn_rl_repo")

from contextlib import ExitStack

import concourse.bass as bass
import concourse.tile as tile
from concourse import bacc, mybir
from concourse.bass_utils import run_bass_kernel_spmd

# Problem constants
B, T, RO, S, _D = 4, 2, 1024, 64, 3
NKP = 512
K = 8
R = 0.12
MAX_SHADING_PTS = 48

N_CORES = 8
NQ_CORE = (B * T * RO * S) // N_CORES  # 65536 queries per core

F32 = mybir.dt.float32
F32R = mybir.dt.float32r
U32 = mybir.dt.uint32


def build_program(nsub, c, nrhs, g, reps=1):
    """Build the Bass program.

    nsub: number of 128-query subtiles per core
    c:    candidate keypoints per subtile
    nrhs: number of rhs matrices (1 = shared by all subtiles, else nsub)
    g:    subtiles per output-staging group (nsub % g == 0)
    reps: repeat the whole computation (timing only)

    Outputs one merged tensor oboth [n, 16] uint32: per query row,
    cols 0:8 = f32 bits of the 8 ascending distances, cols 8:16 = the
    raw candidate indices (uint32).
    """
    assert nsub % g == 0
    n = nsub * 128
    nc = bacc.Bacc("TRN2", target_bir_lowering=False)
    lhs = nc.declare_dram_parameter("lhs", [4, n], F32, isOutput=False)
    rhs = nc.declare_dram_parameter("rhs", [nrhs, 4, c], F32, isOutput=False)
    a2d = nc.declare_dram_parameter("a2d", [128, nsub], F32, isOutput=False)
    oboth = nc.declare_dram_parameter("oboth", [n, 16], U32, isOutput=True)

    with tile.TileContext(nc) as tc, ExitStack() as ctx:
        if reps > 1:
            ctx.enter_context(tc.For_i(0, reps, 1))
        lpool = ctx.enter_context(tc.tile_pool(name="lhs", bufs=2))
        rpool = ctx.enter_context(tc.tile_pool(name="rhs", bufs=2))
        apool = ctx.enter_context(tc.tile_pool(name="a2", bufs=1))
        ppool = ctx.enter_context(tc.tile_pool(name="psum", bufs=6, space="PSUM"))
        dpool = ctx.enter_context(tc.tile_pool(name="d2", bufs=4))
        wpool = ctx.enter_context(tc.tile_pool(name="wide", bufs=2))

        a2all = apool.tile([128, nsub], F32)
        nc.sync.dma_start(out=a2all[:], in_=a2d[:, :])
        rall = None
        if nrhs == 1:
            rall = rpool.tile([4, c], F32)
            nc.sync.dma_start(out=rall[:], in_=rhs[0])

        for m in range(nsub // g):
            wv = wpool.tile([128, g * 8], F32)
            stg = wpool.tile([128, g * 16], F32)
            stg3 = stg[:].rearrange("p (g e) -> p g e", e=16)
            lc = lpool.tile([4, g * 128], F32)
            nc.sync.dma_start(
                out=lc[:], in_=lhs[:, m * g * 128 : (m + 1) * g * 128]
            )
            rc = None
            if nrhs > 1:
                rc = rpool.tile([4, g * c], F32)
                nc.sync.dma_start(
                    out=rc[:].rearrange("f (s c) -> f s c", s=g),
                    in_=rhs[m * g : (m + 1) * g].rearrange("s f c -> f s c"),
                )
            for j in range(g):
                lslice = lc[:, j * 128 : (j + 1) * 128]
                rslice = rall[:] if nrhs == 1 else rc[:, j * c : (j + 1) * c]
                psum = ppool.tile([128, c], F32)
                nc.tensor.matmul(
                    psum[:], lhsT=lslice, rhs=rslice, start=True, stop=True
                )
                d2t = dpool.tile([128, c], F32)
                nc.scalar.copy(out=d2t[:], in_=psum[:])
                nc.vector.max(wv[:, j * 8 : (j + 1) * 8], d2t[:])
                nc.vector.max_index(
                    stg3[:, j, 8:16].bitcast(U32),
                    wv[:, j * 8 : (j + 1) * 8],
                    d2t[:],
                )
            # d2 = a2 - val (clamped to >= 1e-12), dist = sqrt(d2)
            a2x = wpool.tile([128, g * 8], F32)
            a2x3 = a2x[:].rearrange("p (g e) -> p g e", e=8)
            for e in range(8):
                nc.gpsimd.tensor_copy(a2x3[:, :, e], a2all[:, m * g : (m + 1) * g])
            d2n = wpool.tile([128, g * 8], F32)
            # d2n = val - a2 = -(d2);  min with -1e-12  ==  -(max(d2, 1e-12))
            nc.gpsimd.tensor_sub(d2n[:], wv[:], a2x[:])
            nc.gpsimd.tensor_scalar_min(d2n[:], d2n[:], -1e-12)
            nc.scalar.activation(
                stg3[:, :, 0:8],
                d2n[:].rearrange("p (g e) -> p g e", e=8),
                func=mybir.ActivationFunctionType.Sqrt,
                scale=-1.0,
            )
            orows = oboth[m * g * 128 : (m + 1) * g * 128].rearrange(
                "(p g) e -> p (g e)", p=128
            )
            # SWDGE via the (mostly idle) GPSIMD engine: an SP-issued store
            # would hold the SP sequencer while waiting for stg, serializing
            # the next macro's input loads behind it.
            nc.gpsimd.dma_start(out=orows, in_=stg[:].bitcast(U32))
    nc.compile()
    return nc


# ---------------------------------------------------------------------------
# Host-side orchestration
# ---------------------------------------------------------------------------

_PROG_CACHE = {}


def _get_program(nsub, c, nrhs, g):
    key = (nsub, c, nrhs, g)
    if key not in _PROG_CACHE:
        _PROG_CACHE[key] = build_program(nsub, c, nrhs, g)
    return _PROG_CACHE[key]


def _core_inputs_v1(q, kp):
    """Simple dense config: one rhs of all 512 keypoints, identity order.

    q:  [65536, 3] float32 queries of this core
    kp: [512, 3] float32 keypoints of this core's batch
    Returns in_map dict. Device row r maps to query  (s*128 + p)  with
    s = (r // (128*g))*g + r % g,  p = (r // g) % 128.
    """
    nsub, c, g = NQ_CORE // 128, NKP, 64
    lhs = np.empty((4, NQ_CORE), np.float32)
    lhs[:3] = q.T
    lhs[3] = 1.0
    b2 = (kp[:, 0] * kp[:, 0] + kp[:, 1] * kp[:, 1]) + kp[:, 2] * kp[:, 2]
    rhs = np.empty((1, 4, c), np.float32)
    rhs[0, :3] = 2.0 * kp.T
    rhs[0, 3] = -b2
    a2 = (q[:, 0] * q[:, 0] + q[:, 1] * q[:, 1]) + q[:, 2] * q[:, 2]
    a2d = np.ascontiguousarray(a2.reshape(nsub, 128).T)
    return {"lhs": lhs, "rhs": rhs, "a2d": a2d}


def _devrow_to_query(nsub, g):
    """origq[r] for device output row r (v1 ordering)."""
    r = np.arange(nsub * 128)
    m = r // (128 * g)
    j = r % g
    p = (r // g) % 128
    return (m * g + j) * 128 + p


def _devrow_to_slot(nsub, g):
    """slot (= s*128 + p) for device output row r."""
    r = np.arange(nsub * 128)
    m = r // (128 * g)
    j = r % g
    p = (r // g) % 128
    return (m * g + j) * 128 + p


# --- v2: spatial-cell candidate pruning ------------------------------------

D_GRID = 5
C_CAND = 80
G_V2 = 32


def _cell_candidates(kp, d, c):
    """Per-cell rhs [d^3+1, 4, c] and candidate id map [d^3+1, c].

    Cell cc covers box [i,j,l]/d..([i,j,l]+1)/d; candidates are keypoints
    within R+1e-3 of the box. Last row = dummy (all padded) for pad subtiles.
    Pad columns get [0,0,0,-1e30] so their score 2ab-b2 = -1e30 never wins.
    """
    ncell = d**3
    rhs = np.zeros((ncell + 1, 4, c), np.float32)
    rhs[:, 3, :] = -1e30
    cmap = np.zeros((ncell + 1, c), np.int32)
    kp64 = kp.astype(np.float64)
    side = 1.0 / d
    b2 = (kp[:, 0] * kp[:, 0] + kp[:, 1] * kp[:, 1]) + kp[:, 2] * kp[:, 2]
    thr = (R + 1e-3) ** 2
    for i in range(d):
        for j in range(d):
            for l in range(d):
                cc = (i * d + j) * d + l
                lo = np.array([i, j, l]) * side
                dd = np.maximum(np.maximum(lo - kp64, 0), kp64 - (lo + side))
                ids = np.nonzero((dd * dd).sum(1) < thr)[0]
                n = len(ids)
                assert n <= c, f"cell {cc}: {n} candidates > C={c}"
                rhs[cc, 0, :n] = 2.0 * kp[ids, 0]
                rhs[cc, 1, :n] = 2.0 * kp[ids, 1]
                rhs[cc, 2, :n] = 2.0 * kp[ids, 2]
                rhs[cc, 3, :n] = -b2[ids]
                cmap[cc, :n] = ids
    return rhs, cmap


def _assign_subtiles(q, d):
    """Sort queries into cells; chunk each cell into 128-query subtiles.

    Returns (perm2 [nsub_used*128] orig-query index with -1 padding,
             sub_cell [nsub_used] cell id per subtile).
    """
    nq = q.shape[0]
    cid = np.clip((q * d).astype(np.int32), 0, d - 1)
    cell = (cid[:, 0] * d + cid[:, 1]) * d + cid[:, 2]
    order = np.argsort(cell, kind="stable")
    counts = np.bincount(cell, minlength=d**3)
    perm2 = []
    sub_cell = []
    start = 0
    for cc in range(d**3):
        n = int(counts[cc])
        qs = order[start : start + n]
        start += n
        for o in range(0, n, 128):
            chunk = qs[o : o + 128]
            if len(chunk) < 128:
                chunk = np.concatenate(
                    [chunk, np.full(128 - len(chunk), -1, np.int64)]
                )
            perm2.append(chunk)
            sub_cell.append(cc)
    return np.concatenate(perm2), np.asarray(sub_cell, np.int64)


def _core_inputs_v2(q, kp, nsub, c, g):
    """Spatial config inputs + mappings for one core."""
    perm2, sub_cell = _assign_subtiles(q, D_GRID)
    nsub_used = len(sub_cell)
    assert nsub_used <= nsub, f"{nsub_used} subtiles > program NSUB={nsub}"
    perm2 = np.concatenate(
        [perm2, np.full((nsub - nsub_used) * 128, -1, np.int64)]
    )
    sub_cell = np.concatenate(
        [sub_cell, np.full(nsub - nsub_used, D_GRID**3, np.int64)]
    )

    cell_rhs, cell_cmap = _cell_candidates(kp, D_GRID, c)
    rhs = cell_rhs[sub_cell]  # [nsub, 4, c]
    cmap = cell_cmap[sub_cell]  # [nsub, c]

    qsafe = np.where(perm2 >= 0, perm2, 0)
    qc = q[qsafe]  # [nsub*128, 3] slot-ordered coords
    lhs = np.empty((4, nsub * 128), np.float32)
    lhs[:3] = qc.T
    lhs[3] = 1.0
    a2 = (qc[:, 0] * qc[:, 0] + qc[:, 1] * qc[:, 1]) + qc[:, 2] * qc[:, 2]
    a2d = np.ascontiguousarray(a2.reshape(nsub, 128).T)
    return {"lhs": lhs, "rhs": rhs, "a2d": a2d}, perm2, cmap


def kernel_v2(x, kp_pos):
    x = np.asarray(x, dtype=np.float32)
    kp_pos = np.asarray(kp_pos, dtype=np.float32)
    rays = T * RO
    c, g = C_CAND, G_V2

    xq = x.reshape(B, 2, rays // 2 * S, 3)
    preps = []
    nsub_needed = 0
    for core in range(N_CORES):
        b, half = divmod(core, 2)
        perm2, sub_cell = _assign_subtiles(xq[b, half], D_GRID)
        nsub_needed = max(nsub_needed, len(sub_cell))
    nsub = ((nsub_needed + g - 1) // g) * g

    in_maps, perms, cmaps = [], [], []
    for core in range(N_CORES):
        b, half = divmod(core, 2)
        im, perm2, cmap = _core_inputs_v2(xq[b, half], kp_pos[b], nsub, c, g)
        in_maps.append(im)
        perms.append(perm2)
        cmaps.append(cmap)

    nc = _get_program(nsub, c, nsub, g)
    res = run_bass_kernel_spmd(nc, in_maps, core_ids=list(range(N_CORES)))

    slot_of_devrow = _devrow_to_slot(nsub, g)
    devrow_of_slot = np.empty_like(slot_of_devrow)
    devrow_of_slot[slot_of_devrow] = np.arange(slot_of_devrow.size)

    vals = np.empty((B, rays * S, 8), np.float32)
    idx = np.empty((B, rays * S, 8), np.int32)
    half_n = rays // 2 * S
    s_of_slot = np.arange(nsub * 128) // 128
    for core in range(N_CORES):
        b, half = divmod(core, 2)
        ob = res.results[core]["oboth"]
        od = ob.view(np.float32)[:, :8][devrow_of_slot]  # slot order
        oi = ob.view(np.int32)[:, 8:][devrow_of_slot]
        kpid = cmaps[core][s_of_slot[:, None], oi]  # [n, 8]
        perm2 = perms[core]
        valid = perm2 >= 0
        dst = vals[b, half * half_n : (half + 1) * half_n]
        dsti = idx[b, half * half_n : (half + 1) * half_n]
        dst[perm2[valid]] = od[valid]
        dsti[perm2[valid]] = kpid[valid]

    return _postprocess(x, vals, idx)


def kernel(x, kp_pos):
    x = np.asarray(x, dtype=np.float32)
    kp_pos = np.asarray(kp_pos, dtype=np.float32)
    rays = T * RO

    nsub, c, nrhs, g = NQ_CORE // 128, NKP, 1, 64
    nc = _get_program(nsub, c, nrhs, g)

    xq = x.reshape(B, 2, rays // 2 * S, 3)
    in_maps = []
    for core in range(N_CORES):
        b, half = divmod(core, 2)
        in_maps.append(_core_inputs_v1(xq[b, half], kp_pos[b]))

    res = run_bass_kernel_spmd(nc, in_maps, core_ids=list(range(N_CORES)))

    perm = _devrow_to_query(nsub, g)
    inv = np.empty_like(perm)
    inv[perm] = np.arange(perm.size)

    vals = np.empty((B, rays * S, 8), np.float32)
    idx = np.empty((B, rays * S, 8), np.int32)
    half_n = rays // 2 * S
    for core in range(N_CORES):
        b, half = divmod(core, 2)
        ob = res.results[core]["oboth"]
        od = ob.view(np.float32)[:, :8]
        oi = ob.view(np.int32)[:, 8:]
        vals[b, half * half_n : (half + 1) * half_n] = od[inv]
        idx[b, half * half_n : (half + 1) * half_n] = oi[inv]

    return _postprocess(x, vals, idx)


def _postprocess(x, vals, idx):
    """vals: [B, rays*S, 8] ascending distances; idx: keypoint ids (0..511)."""
    rays = T * RO
    vals = vals.reshape(B, rays, S, 8)
    idx = idx.reshape(B, rays, S, 8)

    valid_nb = vals < R
    offset = (NKP * np.arange(B, dtype=np.int32)).reshape(B, 1, 1, 1)
    nb_idx = np.where(valid_nb, idx + offset, -1).astype(np.int32)

    valid_pts = valid_nb[..., 0:1]  # any() == slot 0 since ascending
    csum = np.cumsum(valid_pts.astype(np.int32), axis=-2)
    valid_pts = np.logical_and(valid_pts, csum <= MAX_SHADING_PTS)

    nb_idx = np.where(valid_pts, nb_idx, -1)
    nb_dist = np.where(np.logical_and(valid_pts, valid_nb), vals, 0.0).astype(
        np.float32
    )
    shading = np.where(valid_pts, x.reshape(B, rays, S, 3), 0.0).astype(np.float32)

    num_valid = valid_pts.sum(axis=-2, keepdims=True)
    mask = np.arange(MAX_SHADING_PTS).reshape(1, 1, -1, 1) < num_valid

    return (
        nb_idx.reshape(B, T, RO, S, K),
        shading.reshape(B, T, RO, S, 3),
        nb_dist.reshape(B, T, RO, S, K),
        mask.reshape(B, T, RO, MAX_SHADING_PTS, 1),
    )


# revision 18
# speedup vs baseline: 1.0217x; 1.0203x over previous
"""Trainium2 Bass kernel for nn_Aggregator (retrieval_knn).

Reference computation: for each of B*T*Ro*S = 524288 query points, find the
8 nearest of 512 keypoints (per batch), threshold at R=0.12, cap at 48 valid
points per ray (64 points), emit (neighbor_idx, shading_pts, neighbor_dist,
mask).

Device part (per core, SPMD over 8 cores; core = (batch, ray-half)):
  - PE matmul computes s[q,m] = 2*q.k - |k|^2  (= a2[q] - d2[q,m]) for a
    128-query subtile against C candidate keypoints.
  - DVE max/max_index extract the top-8 values (= 8 smallest d2) + indices.
  - d2 = a2 - val, clamped, sqrt -> 8 ascending distances per query.
Host part: radius mask, per-ray cumsum cap, -1/0 fills, mask construction
(cheap O(N) numpy, exactly mirroring the reference semantics).
"""

import os
import sys

import numpy as np

sys.path.insert(0, "/opt/trn_rl_repo")

from contextlib import ExitStack

import concourse.bass as bass
import concourse.tile as tile
from concourse import bacc, mybir
from concourse.bass_utils import run_bass_kernel_spmd

# Problem constants
B, T, RO, S, _D = 4, 2, 1024, 64, 3
NKP = 512
K = 8
R = 0.12
MAX_SHADING_PTS = 48

N_CORES = 8
NQ_CORE = (B * T * RO * S) // N_CORES  # 65536 queries per core

F32 = mybir.dt.float32
F32R = mybir.dt.float32r
U32 = mybir.dt.uint32


def build_program(nsub, c, nrhs, g, reps=1, bufs=(2, 2, 6, 4, 3)):
    """Build the Bass program.

    nsub: number of 128-query subtiles per core
    c:    candidate keypoints per subtile
    nrhs: number of rhs matrices (1 = shared by all subtiles, else nsub)
    g:    subtiles per output-staging group (nsub % g == 0)
    reps: repeat the whole computation (timing only)

    Outputs one merged tensor oboth [n, 16] uint32: per query row,
    cols 0:8 = f32 bits of the 8 ascending distances, cols 8:16 = the
    raw candidate indices (uint32).
    """
    assert nsub % g == 0
    lb, rb, pb, db, wb = bufs
    n = nsub * 128
    nc = bacc.Bacc("TRN2", target_bir_lowering=False)
    lhs = nc.declare_dram_parameter("lhs", [4, n], F32, isOutput=False)
    rhs = nc.declare_dram_parameter("rhs", [nrhs, 4, c], F32, isOutput=False)
    a2d = nc.declare_dram_parameter("a2d", [128, nsub], F32, isOutput=False)
    oboth = nc.declare_dram_parameter("oboth", [n, 16], U32, isOutput=True)

    with tile.TileContext(nc) as tc, ExitStack() as ctx:
        if reps > 1:
            ctx.enter_context(tc.For_i(0, reps, 1))
        lpool = ctx.enter_context(tc.tile_pool(name="lhs", bufs=lb))
        rpool = ctx.enter_context(tc.tile_pool(name="rhs", bufs=rb))
        apool = ctx.enter_context(tc.tile_pool(name="a2", bufs=1))
        ppool = ctx.enter_context(tc.tile_pool(name="psum", bufs=pb, space="PSUM"))
        dpool = ctx.enter_context(tc.tile_pool(name="d2", bufs=db))
        wpool = ctx.enter_context(tc.tile_pool(name="wide", bufs=wb))

        a2all = apool.tile([128, nsub], F32)
        nc.sync.dma_start(out=a2all[:], in_=a2d[:, :])
        rall = None
        if nrhs == 1:
            rall = rpool.tile([4, c], F32)
            nc.sync.dma_start(out=rall[:], in_=rhs[0])

        def make_post(m, wv, stg, stg3):
            # d2 = a2 - val (clamped to >= 1e-12), dist = sqrt(d2).
            # Emitted split into the NEXT macro's subtile loop so no
            # engine's sequencer blocks at a macro boundary waiting for
            # the whole previous macro to finish.
            def pool_part():
                a2x = wpool.tile([128, g * 8], F32)
                a2x3 = a2x[:].rearrange("p (g e) -> p g e", e=8)
                for e in range(8):
                    nc.gpsimd.tensor_copy(
                        a2x3[:, :, e], a2all[:, m * g : (m + 1) * g]
                    )
                # stg dist cols = val - a2 = -(d2); min -1e-12 == -max(d2,eps)
                sd = stg3[:, :, 0:8]
                nc.gpsimd.tensor_sub(
                    sd, wv[:].rearrange("p (g e) -> p g e", e=8), a2x3
                )
                nc.gpsimd.tensor_scalar_min(sd, sd, -1e-12)

            def act_part():
                sd = stg3[:, :, 0:8]
                nc.scalar.activation(
                    sd, sd, func=mybir.ActivationFunctionType.Sqrt, scale=-1.0
                )

            def dma_part():
                orows = oboth[m * g * 128 : (m + 1) * g * 128].rearrange(
                    "(p g) e -> p (g e)", p=128
                )
                # SWDGE via the (mostly idle) GPSIMD engine: an SP-issued
                # store would hold the SP sequencer while waiting for stg.
                nc.gpsimd.dma_start(out=orows, in_=stg[:].bitcast(U32))

            return pool_part, act_part, dma_part

        post = None
        for m in range(nsub // g):
            wv = wpool.tile([128, g * 8], F32)
            stg = wpool.tile([128, g * 16], F32)
            stg3 = stg[:].rearrange("p (g e) -> p g e", e=16)
            lc = lpool.tile([4, g * 128], F32)
            nc.sync.dma_start(
                out=lc[:], in_=lhs[:, m * g * 128 : (m + 1) * g * 128]
            )
            rc = None
            if nrhs > 1:
                rc = rpool.tile([4, g * c], F32)
                nc.sync.dma_start(
                    out=rc[:].rearrange("f (s c) -> f s c", s=g),
                    in_=rhs[m * g : (m + 1) * g].rearrange("s f c -> f s c"),
                )
            p0 = g // 4
            p1 = max(p0 + 1, 2 * g // 3)
            p2 = min(max(p1 + 1, g - 2), g - 1)
            for j in range(g):
                if post is not None:
                    if j == p0:
                        post[0]()
                    elif j == p1:
                        post[1]()
                    elif j == p2:
                        post[2]()
                lslice = lc[:, j * 128 : (j + 1) * 128]
                rslice = rall[:] if nrhs == 1 else rc[:, j * c : (j + 1) * c]
                psum = ppool.tile([128, c], F32)
                nc.tensor.matmul(
                    psum[:], lhsT=lslice, rhs=rslice, start=True, stop=True
                )
                d2t = dpool.tile([128, c], F32)
                nc.scalar.copy(out=d2t[:], in_=psum[:])
                nc.vector.max(wv[:, j * 8 : (j + 1) * 8], d2t[:])
                nc.vector.max_index(
                    stg3[:, j, 8:16].bitcast(U32),
                    wv[:, j * 8 : (j + 1) * 8],
                    d2t[:],
                )
            post = make_post(m, wv, stg, stg3)
        for part in post:
            part()
    nc.compile()
    return nc


# ---------------------------------------------------------------------------
# Host-side orchestration
# ---------------------------------------------------------------------------

_PROG_CACHE = {}


def _get_program(nsub, c, nrhs, g):
    key = (nsub, c, nrhs, g)
    if key not in _PROG_CACHE:
        _PROG_CACHE[key] = build_program(nsub, c, nrhs, g)
    return _PROG_CACHE[key]


def _core_inputs_v1(q, kp):
    """Simple dense config: one rhs of all 512 keypoints, identity order.

    q:  [65536, 3] float32 queries of this core
    kp: [512, 3] float32 keypoints of this core's batch
    Returns in_map dict. Device row r maps to query  (s*128 + p)  with
    s = (r // (128*g))*g + r % g,  p = (r // g) % 128.
    """
    nsub, c, g = NQ_CORE // 128, NKP, 64
    lhs = np.empty((4, NQ_CORE), np.float32)
    lhs[:3] = q.T
    lhs[3] = 1.0
    b2 = (kp[:, 0] * kp[:, 0] + kp[:, 1] * kp[:, 1]) + kp[:, 2] * kp[:, 2]
    rhs = np.empty((1, 4, c), np.float32)
    rhs[0, :3] = 2.0 * kp.T
    rhs[0, 3] = -b2
    a2 = (q[:, 0] * q[:, 0] + q[:, 1] * q[:, 1]) + q[:, 2] * q[:, 2]
    a2d = np.ascontiguousarray(a2.reshape(nsub, 128).T)
    return {"lhs": lhs, "rhs": rhs, "a2d": a2d}


def _devrow_to_query(nsub, g):
    """origq[r] for device output row r (v1 ordering)."""
    r = np.arange(nsub * 128)
    m = r // (128 * g)
    j = r % g
    p = (r // g) % 128
    return (m * g + j) * 128 + p


def _devrow_to_slot(nsub, g):
    """slot (= s*128 + p) for device output row r."""
    r = np.arange(nsub * 128)
    m = r // (128 * g)
    j = r % g
    p = (r // g) % 128
    return (m * g + j) * 128 + p


# --- v2: spatial-cell candidate pruning ------------------------------------

D_GRID = 5
C_CAND = 80
G_V2 = 32


def _cell_candidates(kp, d, c):
    """Per-cell rhs [d^3+1, 4, c] and candidate id map [d^3+1, c].

    Cell cc covers box [i,j,l]/d..([i,j,l]+1)/d; candidates are keypoints
    within R+1e-3 of the box. Last row = dummy (all padded) for pad subtiles.
    Pad columns get [0,0,0,-1e30] so their score 2ab-b2 = -1e30 never wins.
    """
    ncell = d**3
    rhs = np.zeros((ncell + 1, 4, c), np.float32)
    rhs[:, 3, :] = -1e30
    cmap = np.zeros((ncell + 1, c), np.int32)
    kp64 = kp.astype(np.float64)
    side = 1.0 / d
    b2 = (kp[:, 0] * kp[:, 0] + kp[:, 1] * kp[:, 1]) + kp[:, 2] * kp[:, 2]
    thr = (R + 1e-3) ** 2
    for i in range(d):
        for j in range(d):
            for l in range(d):
                cc = (i * d + j) * d + l
                lo = np.array([i, j, l]) * side
                dd = np.maximum(np.maximum(lo - kp64, 0), kp64 - (lo + side))
                ids = np.nonzero((dd * dd).sum(1) < thr)[0]
                n = len(ids)
                assert n <= c, f"cell {cc}: {n} candidates > C={c}"
                rhs[cc, 0, :n] = 2.0 * kp[ids, 0]
                rhs[cc, 1, :n] = 2.0 * kp[ids, 1]
                rhs[cc, 2, :n] = 2.0 * kp[ids, 2]
                rhs[cc, 3, :n] = -b2[ids]
                cmap[cc, :n] = ids
    return rhs, cmap


def _assign_subtiles(q, d):
    """Sort queries into cells; chunk each cell into 128-query subtiles.

    Returns (perm2 [nsub_used*128] orig-query index with -1 padding,
             sub_cell [nsub_used] cell id per subtile).
    """
    nq = q.shape[0]
    cid = np.clip((q * d).astype(np.int32), 0, d - 1)
    cell = (cid[:, 0] * d + cid[:, 1]) * d + cid[:, 2]
    order = np.argsort(cell, kind="stable")
    counts = np.bincount(cell, minlength=d**3)
    perm2 = []
    sub_cell = []
    start = 0
    for cc in range(d**3):
        n = int(counts[cc])
        qs = order[start : start + n]
        start += n
        for o in range(0, n, 128):
            chunk = qs[o : o + 128]
            if len(chunk) < 128:
                chunk = np.concatenate(
                    [chunk, np.full(128 - len(chunk), -1, np.int64)]
                )
            perm2.append(chunk)
            sub_cell.append(cc)
    return np.concatenate(perm2), np.asarray(sub_cell, np.int64)


def _core_inputs_v2(q, kp, nsub, c, g):
    """Spatial config inputs + mappings for one core."""
    perm2, sub_cell = _assign_subtiles(q, D_GRID)
    nsub_used = len(sub_cell)
    assert nsub_used <= nsub, f"{nsub_used} subtiles > program NSUB={nsub}"
    perm2 = np.concatenate(
        [perm2, np.full((nsub - nsub_used) * 128, -1, np.int64)]
    )
    sub_cell = np.concatenate(
        [sub_cell, np.full(nsub - nsub_used, D_GRID**3, np.int64)]
    )

    cell_rhs, cell_cmap = _cell_candidates(kp, D_GRID, c)
    rhs = cell_rhs[sub_cell]  # [nsub, 4, c]
    cmap = cell_cmap[sub_cell]  # [nsub, c]

    qsafe = np.where(perm2 >= 0, perm2, 0)
    qc = q[qsafe]  # [nsub*128, 3] slot-ordered coords
    lhs = np.empty((4, nsub * 128), np.float32)
    lhs[:3] = qc.T
    lhs[3] = 1.0
    a2 = (qc[:, 0] * qc[:, 0] + qc[:, 1] * qc[:, 1]) + qc[:, 2] * qc[:, 2]
    a2d = np.ascontiguousarray(a2.reshape(nsub, 128).T)
    return {"lhs": lhs, "rhs": rhs, "a2d": a2d}, perm2, cmap


def kernel_v2(x, kp_pos):
    x = np.asarray(x, dtype=np.float32)
    kp_pos = np.asarray(kp_pos, dtype=np.float32)
    rays = T * RO
    c, g = C_CAND, G_V2

    xq = x.reshape(B, 2, rays // 2 * S, 3)
    preps = []
    nsub_needed = 0
    for core in range(N_CORES):
        b, half = divmod(core, 2)
        perm2, sub_cell = _assign_subtiles(xq[b, half], D_GRID)
        nsub_needed = max(nsub_needed, len(sub_cell))
    nsub = ((nsub_needed + g - 1) // g) * g

    in_maps, perms, cmaps = [], [], []
    for core in range(N_CORES):
        b, half = divmod(core, 2)
        im, perm2, cmap = _core_inputs_v2(xq[b, half], kp_pos[b], nsub, c, g)
        in_maps.append(im)
        perms.append(perm2)
        cmaps.append(cmap)

    nc = _get_program(nsub, c, nsub, g)
    res = run_bass_kernel_spmd(nc, in_maps, core_ids=list(range(N_CORES)))

    slot_of_devrow = _devrow_to_slot(nsub, g)
    devrow_of_slot = np.empty_like(slot_of_devrow)
    devrow_of_slot[slot_of_devrow] = np.arange(slot_of_devrow.size)

    vals = np.empty((B, rays * S, 8), np.float32)
    idx = np.empty((B, rays * S, 8), np.int32)
    half_n = rays // 2 * S
    s_of_slot = np.arange(nsub * 128) // 128
    for core in range(N_CORES):
        b, half = divmod(core, 2)
        ob = res.results[core]["oboth"]
        od = ob.view(np.float32)[:, :8][devrow_of_slot]  # slot order
        oi = ob.view(np.int32)[:, 8:][devrow_of_slot]
        kpid = cmaps[core][s_of_slot[:, None], oi]  # [n, 8]
        perm2 = perms[core]
        valid = perm2 >= 0
        dst = vals[b, half * half_n : (half + 1) * half_n]
        dsti = idx[b, half * half_n : (half + 1) * half_n]
        dst[perm2[valid]] = od[valid]
        dsti[perm2[valid]] = kpid[valid]

    return _postprocess(x, vals, idx)


def kernel(x, kp_pos):
    x = np.asarray(x, dtype=np.float32)
    kp_pos = np.asarray(kp_pos, dtype=np.float32)
    rays = T * RO

    nsub, c, nrhs, g = NQ_CORE // 128, NKP, 1, 64
    nc = _get_program(nsub, c, nrhs, g)

    xq = x.reshape(B, 2, rays // 2 * S, 3)
    in_maps = []
    for core in range(N_CORES):
        b, half = divmod(core, 2)
        in_maps.append(_core_inputs_v1(xq[b, half], kp_pos[b]))

    res = run_bass_kernel_spmd(nc, in_maps, core_ids=list(range(N_CORES)))

    perm = _devrow_to_query(nsub, g)
    inv = np.empty_like(perm)
    inv[perm] = np.arange(perm.size)

    vals = np.empty((B, rays * S, 8), np.float32)
    idx = np.empty((B, rays * S, 8), np.int32)
    half_n = rays // 2 * S
    for core in range(N_CORES):
        b, half = divmod(core, 2)
        ob = res.results[core]["oboth"]
        od = ob.view(np.float32)[:, :8]
        oi = ob.view(np.int32)[:, 8:]
        vals[b, half * half_n : (half + 1) * half_n] = od[inv]
        idx[b, half * half_n : (half + 1) * half_n] = oi[inv]

    return _postprocess(x, vals, idx)


def _postprocess(x, vals, idx):
    """vals: [B, rays*S, 8] ascending distances; idx: keypoint ids (0..511)."""
    rays = T * RO
    vals = vals.reshape(B, rays, S, 8)
    idx = idx.reshape(B, rays, S, 8)

    valid_nb = vals < R
    offset = (NKP * np.arange(B, dtype=np.int32)).reshape(B, 1, 1, 1)
    nb_idx = np.where(valid_nb, idx + offset, -1).astype(np.int32)

    valid_pts = valid_nb[..., 0:1]  # any() == slot 0 since ascending
    csum = np.cumsum(valid_pts.astype(np.int32), axis=-2)
    valid_pts = np.logical_and(valid_pts, csum <= MAX_SHADING_PTS)

    nb_idx = np.where(valid_pts, nb_idx, -1)
    nb_dist = np.where(np.logical_and(valid_pts, valid_nb), vals, 0.0).astype(
        np.float32
    )
    shading = np.where(valid_pts, x.reshape(B, rays, S, 3), 0.0).astype(np.float32)

    num_valid = valid_pts.sum(axis=-2, keepdims=True)
    mask = np.arange(MAX_SHADING_PTS).reshape(1, 1, -1, 1) < num_valid

    return (
        nb_idx.reshape(B, T, RO, S, K),
        shading.reshape(B, T, RO, S, 3),
        nb_dist.reshape(B, T, RO, S, K),
        mask.reshape(B, T, RO, MAX_SHADING_PTS, 1),
    )


# revision 25
# speedup vs baseline: 1.3524x; 1.3238x over previous
"""Trainium2 Bass kernel for nn_Aggregator (retrieval_knn).

Reference computation: for each of B*T*Ro*S = 524288 query points, find the
8 nearest of 512 keypoints (per batch), threshold at R=0.12, cap at 48 valid
points per ray (64 points), emit (neighbor_idx, shading_pts, neighbor_dist,
mask).

Device part (per core, SPMD over 8 cores; core = (batch, ray-half)):
  - PE matmul computes s[q,m] = 2*q.k - |k|^2  (= a2[q] - d2[q,m]) for a
    128-query subtile against C candidate keypoints.
  - DVE max/max_index extract the top-8 values (= 8 smallest d2) + indices.
  - d2 = a2 - val, clamped, sqrt -> 8 ascending distances per query.
Host part: radius mask, per-ray cumsum cap, -1/0 fills, mask construction
(cheap O(N) numpy, exactly mirroring the reference semantics).
"""

import os
import sys

import numpy as np

sys.path.insert(0, "/opt/trn_rl_repo")

from contextlib import ExitStack

import concourse.bass as bass
import concourse.tile as tile
from concourse import bacc, mybir
from concourse.bass_utils import run_bass_kernel_spmd

# Problem constants
B, T, RO, S, _D = 4, 2, 1024, 64, 3
NKP = 512
K = 8
R = 0.12
MAX_SHADING_PTS = 48

N_CORES = 8
NQ_CORE = (B * T * RO * S) // N_CORES  # 65536 queries per core

F32 = mybir.dt.float32
F32R = mybir.dt.float32r
U32 = mybir.dt.uint32


def build_program(nsub, c, nrhs, g, reps=1, bufs=(2, 2, 6, 4, 3)):
    """Build the Bass program.

    nsub: number of 128-query subtiles per core
    c:    candidate keypoints per subtile
    nrhs: number of rhs matrices (1 = shared by all subtiles, else nsub)
    g:    subtiles per output-staging group (nsub % g == 0)
    reps: repeat the whole computation (timing only)

    Outputs one merged tensor oboth [n, 16] uint32: per query row,
    cols 0:8 = f32 bits of the 8 ascending distances, cols 8:16 = the
    raw candidate indices (uint32).
    """
    assert nsub % g == 0
    lb, rb, pb, db, wb = bufs
    n = nsub * 128
    nc = bacc.Bacc("TRN2", target_bir_lowering=False)
    lhs = nc.declare_dram_parameter("lhs", [4, n], F32, isOutput=False)
    rhs = nc.declare_dram_parameter("rhs", [nrhs, 4, c], F32, isOutput=False)
    a2d = nc.declare_dram_parameter("a2d", [128, nsub], F32, isOutput=False)
    oboth = nc.declare_dram_parameter("oboth", [n, 16], U32, isOutput=True)

    with tile.TileContext(nc) as tc, ExitStack() as ctx:
        if reps > 1:
            ctx.enter_context(tc.For_i(0, reps, 1))
        lpool = ctx.enter_context(tc.tile_pool(name="lhs", bufs=lb))
        rpool = ctx.enter_context(tc.tile_pool(name="rhs", bufs=rb))
        apool = ctx.enter_context(tc.tile_pool(name="a2", bufs=1))
        ppool = ctx.enter_context(tc.tile_pool(name="psum", bufs=pb, space="PSUM"))
        dpool = ctx.enter_context(tc.tile_pool(name="d2", bufs=db))
        wpool = ctx.enter_context(tc.tile_pool(name="wide", bufs=wb))

        a2all = apool.tile([128, nsub], F32)
        nc.sync.dma_start(out=a2all[:], in_=a2d[:, :])
        rall = None
        if nrhs == 1:
            rall = rpool.tile([4, c], F32)
            nc.sync.dma_start(out=rall[:], in_=rhs[0])

        skip_post = os.environ.get("SKIP_POST", "")

        def make_post(m, wv, stg, stg3):
            # d2 = a2 - val (clamped to >= 1e-12), dist = sqrt(d2).
            # Emitted split into the NEXT macro's subtile loop so no
            # engine's sequencer blocks at a macro boundary waiting for
            # the whole previous macro to finish.
            def pool_part():
                if skip_post == "all":
                    return
                _we = nc.gpsimd
                a2x = wpool.tile([128, g * 8], F32)
                a2x3 = a2x[:].rearrange("p (g e) -> p g e", e=8)
                for e in range(8):
                    _we.tensor_copy(
                        a2x3[:, :, e], a2all[:, m * g : (m + 1) * g]
                    )
                # stg dist cols = val - a2 = -(d2); min -1e-12 == -max(d2,eps)
                sd = stg3[:, :, 0:8]
                _we.tensor_sub(
                    sd, wv[:].rearrange("p (g e) -> p g e", e=8), a2x3
                )
                _we.tensor_scalar_min(sd, sd, -1e-12)

            def act_part():
                if skip_post == "all":
                    return
                sd = stg3[:, :, 0:8]
                nc.scalar.activation(
                    sd, sd, func=mybir.ActivationFunctionType.Sqrt, scale=-1.0
                )

            def dma_part():
                if skip_post in ("all", "dma"):
                    # still need an output write so tiles release; write stg raw
                    pass
                orows = oboth[m * g * 128 : (m + 1) * g * 128].rearrange(
                    "(p g) e -> p (g e)", p=128
                )
                _ob_eng = nc.sync
                _ob_eng.dma_start(out=orows, in_=stg[:].bitcast(U32))

            return pool_part, act_part, dma_part

        post = None
        for m in range(nsub // g):
            wv = wpool.tile([128, g * 8], F32)
            stg = wpool.tile([128, g * 16], F32)
            stg3 = stg[:].rearrange("p (g e) -> p g e", e=16)
            lc = lpool.tile([4, g * 128], F32)
            nc.sync.dma_start(
                out=lc[:], in_=lhs[:, m * g * 128 : (m + 1) * g * 128]
            )
            rc = None
            if nrhs > 1:
                rc = rpool.tile([4, g * c], F32)
                _rc_eng = nc.gpsimd  # SWDGE: keeps the SP sequencer free
                _rc_eng.dma_start(
                    out=rc[:].rearrange("f (s c) -> f s c", s=g),
                    in_=rhs[m * g : (m + 1) * g].rearrange("s f c -> f s c"),
                )
            p0 = g // 4
            p1 = max(p0 + 1, 2 * g // 3)
            p2 = min(max(p1 + 1, g - 2), g - 1)
            for j in range(g):
                if post is not None:
                    if j == p0:
                        post[0]()
                    elif j == p1:
                        post[1]()
                    elif j == p2:
                        post[2]()
                lslice = lc[:, j * 128 : (j + 1) * 128]
                rslice = rall[:] if nrhs == 1 else rc[:, j * c : (j + 1) * c]
                psum = ppool.tile([128, c], F32)
                nc.tensor.matmul(
                    psum[:], lhsT=lslice, rhs=rslice, start=True, stop=True
                )
                d2t = dpool.tile([128, c], F32)
                nc.scalar.copy(out=d2t[:], in_=psum[:])
                nc.vector.max(wv[:, j * 8 : (j + 1) * 8], d2t[:])
                nc.vector.max_index(
                    stg3[:, j, 8:16].bitcast(U32),
                    wv[:, j * 8 : (j + 1) * 8],
                    d2t[:],
                )
            post = make_post(m, wv, stg, stg3)
        for part in post:
            part()
    nc.compile()
    return nc


# ---------------------------------------------------------------------------
# Host-side orchestration
# ---------------------------------------------------------------------------

_PROG_CACHE = {}


def _get_program(nsub, c, nrhs, g):
    key = (nsub, c, nrhs, g)
    if key not in _PROG_CACHE:
        _PROG_CACHE[key] = build_program(nsub, c, nrhs, g)
    return _PROG_CACHE[key]


def _core_inputs_v1(q, kp):
    """Simple dense config: one rhs of all 512 keypoints, identity order.

    q:  [65536, 3] float32 queries of this core
    kp: [512, 3] float32 keypoints of this core's batch
    Returns in_map dict. Device row r maps to query  (s*128 + p)  with
    s = (r // (128*g))*g + r % g,  p = (r // g) % 128.
    """
    nsub, c, g = NQ_CORE // 128, NKP, 64
    lhs = np.empty((4, NQ_CORE), np.float32)
    lhs[:3] = q.T
    lhs[3] = 1.0
    b2 = (kp[:, 0] * kp[:, 0] + kp[:, 1] * kp[:, 1]) + kp[:, 2] * kp[:, 2]
    rhs = np.empty((1, 4, c), np.float32)
    rhs[0, :3] = 2.0 * kp.T
    rhs[0, 3] = -b2
    a2 = (q[:, 0] * q[:, 0] + q[:, 1] * q[:, 1]) + q[:, 2] * q[:, 2]
    a2d = np.ascontiguousarray(a2.reshape(nsub, 128).T)
    return {"lhs": lhs, "rhs": rhs, "a2d": a2d}


def _devrow_to_query(nsub, g):
    """origq[r] for device output row r (v1 ordering)."""
    r = np.arange(nsub * 128)
    m = r // (128 * g)
    j = r % g
    p = (r // g) % 128
    return (m * g + j) * 128 + p


def _devrow_to_slot(nsub, g):
    """slot (= s*128 + p) for device output row r."""
    r = np.arange(nsub * 128)
    m = r // (128 * g)
    j = r % g
    p = (r // g) % 128
    return (m * g + j) * 128 + p


# --- v2: spatial-cell candidate pruning ------------------------------------

D_GRID = 4
C_CAND = 72
G_V2 = 32


def _cell_cand_ids(kp, d):
    """Candidate keypoint id lists per cell (within R+1e-3 of the cell box)."""
    kp64 = kp.astype(np.float64)
    side = 1.0 / d
    thr = (R + 1e-3) ** 2
    out = []
    for i in range(d):
        for j in range(d):
            for l in range(d):
                lo = np.array([i, j, l]) * side
                dd = np.maximum(np.maximum(lo - kp64, 0), kp64 - (lo + side))
                out.append(np.nonzero((dd * dd).sum(1) < thr)[0])
    return out


def _cell_candidates(kp, cand_ids, c):
    """Per-cell rhs [d^3+1, 4, c] and candidate id map [d^3+1, c].

    Last row = dummy (all padded) for pad subtiles. Pad columns get
    [0,0,0,-1e30] so their score 2ab-b2 = -1e30 never wins.
    """
    ncell = len(cand_ids)
    rhs = np.zeros((ncell + 1, 4, c), np.float32)
    rhs[:, 3, :] = -1e30
    cmap = np.zeros((ncell + 1, c), np.int32)
    b2 = (kp[:, 0] * kp[:, 0] + kp[:, 1] * kp[:, 1]) + kp[:, 2] * kp[:, 2]
    for cc, ids in enumerate(cand_ids):
        n = len(ids)
        assert n <= c, f"cell {cc}: {n} candidates > C={c}"
        rhs[cc, 0, :n] = 2.0 * kp[ids, 0]
        rhs[cc, 1, :n] = 2.0 * kp[ids, 1]
        rhs[cc, 2, :n] = 2.0 * kp[ids, 2]
        rhs[cc, 3, :n] = -b2[ids]
        cmap[cc, :n] = ids
    return rhs, cmap


def _assign_subtiles(q, d):
    """Sort queries into cells; chunk each cell into 128-query subtiles.

    Returns (perm2 [nsub_used*128] orig-query index with -1 padding,
             sub_cell [nsub_used] cell id per subtile).
    """
    nq = q.shape[0]
    cid = np.clip((q * d).astype(np.int32), 0, d - 1)
    cell = (cid[:, 0] * d + cid[:, 1]) * d + cid[:, 2]
    order = np.argsort(cell, kind="stable")
    counts = np.bincount(cell, minlength=d**3)
    perm2 = []
    sub_cell = []
    start = 0
    for cc in range(d**3):
        n = int(counts[cc])
        qs = order[start : start + n]
        start += n
        for o in range(0, n, 128):
            chunk = qs[o : o + 128]
            if len(chunk) < 128:
                chunk = np.concatenate(
                    [chunk, np.full(128 - len(chunk), -1, np.int64)]
                )
            perm2.append(chunk)
            sub_cell.append(cc)
    return np.concatenate(perm2), np.asarray(sub_cell, np.int64)


def _core_inputs_v2(q, cell_rhs, cell_cmap, nsub, c, g):
    """Spatial config inputs + mappings for one core."""
    perm2, sub_cell = _assign_subtiles(q, D_GRID)
    nsub_used = len(sub_cell)
    assert nsub_used <= nsub, f"{nsub_used} subtiles > program NSUB={nsub}"
    perm2 = np.concatenate(
        [perm2, np.full((nsub - nsub_used) * 128, -1, np.int64)]
    )
    sub_cell = np.concatenate(
        [sub_cell, np.full(nsub - nsub_used, D_GRID**3, np.int64)]
    )

    rhs = cell_rhs[sub_cell]  # [nsub, 4, c]
    cmap = cell_cmap[sub_cell]  # [nsub, c]

    qsafe = np.where(perm2 >= 0, perm2, 0)
    qc = q[qsafe]  # [nsub*128, 3] slot-ordered coords
    lhs = np.empty((4, nsub * 128), np.float32)
    lhs[:3] = qc.T
    lhs[3] = 1.0
    a2 = (qc[:, 0] * qc[:, 0] + qc[:, 1] * qc[:, 1]) + qc[:, 2] * qc[:, 2]
    a2d = np.ascontiguousarray(a2.reshape(nsub, 128).T)
    return {"lhs": lhs, "rhs": rhs, "a2d": a2d}, perm2, cmap


def _prep_v2(x, kp_pos):
    rays = T * RO
    c, g = C_CAND, G_V2
    xq = x.reshape(B, 2, rays // 2 * S, 3)
    nsub_needed = 0
    for core in range(N_CORES):
        b, half = divmod(core, 2)
        perm2, sub_cell = _assign_subtiles(xq[b, half], D_GRID)
        nsub_needed = max(nsub_needed, len(sub_cell))
    nsub = ((nsub_needed + g - 1) // g) * g

    batch_cands = [_cell_cand_ids(kp_pos[b], D_GRID) for b in range(B)]
    c_needed = max(max(len(ids) for ids in bc) for bc in batch_cands)
    c = max(c, -((-c_needed) // 4) * 4)
    batch_rhs = [
        _cell_candidates(kp_pos[b], batch_cands[b], c) for b in range(B)
    ]

    in_maps, perms, cmaps = [], [], []
    for core in range(N_CORES):
        b, half = divmod(core, 2)
        cell_rhs, cell_cmap = batch_rhs[b]
        im, perm2, cmap = _core_inputs_v2(
            xq[b, half], cell_rhs, cell_cmap, nsub, c, g
        )
        in_maps.append(im)
        perms.append(perm2)
        cmaps.append(cmap)
    return nsub, c, g, in_maps, perms, cmaps


def kernel_v2(x, kp_pos):
    x = np.asarray(x, dtype=np.float32)
    kp_pos = np.asarray(kp_pos, dtype=np.float32)
    rays = T * RO
    nsub, c, g, in_maps, perms, cmaps = _prep_v2(x, kp_pos)

    nc = _get_program(nsub, c, nsub, g)
    res = run_bass_kernel_spmd(nc, in_maps, core_ids=list(range(N_CORES)))

    slot_of_devrow = _devrow_to_slot(nsub, g)
    devrow_of_slot = np.empty_like(slot_of_devrow)
    devrow_of_slot[slot_of_devrow] = np.arange(slot_of_devrow.size)

    vals = np.empty((B, rays * S, 8), np.float32)
    idx = np.empty((B, rays * S, 8), np.int32)
    half_n = rays // 2 * S
    s_of_slot = np.arange(nsub * 128) // 128
    for core in range(N_CORES):
        b, half = divmod(core, 2)
        ob = res.results[core]["oboth"]
        od = ob.view(np.float32)[:, :8][devrow_of_slot]  # slot order
        oi = ob.view(np.int32)[:, 8:][devrow_of_slot]
        kpid = cmaps[core][s_of_slot[:, None], oi]  # [n, 8]
        perm2 = perms[core]
        valid = perm2 >= 0
        dst = vals[b, half * half_n : (half + 1) * half_n]
        dsti = idx[b, half * half_n : (half + 1) * half_n]
        dst[perm2[valid]] = od[valid]
        dsti[perm2[valid]] = kpid[valid]

    return _postprocess(x, vals, idx)


def kernel(x, kp_pos):
    x = np.asarray(x, dtype=np.float32)
    kp_pos = np.asarray(kp_pos, dtype=np.float32)
    rays = T * RO

    nsub, c, nrhs, g = NQ_CORE // 128, NKP, 1, 64
    nc = _get_program(nsub, c, nrhs, g)

    xq = x.reshape(B, 2, rays // 2 * S, 3)
    in_maps = []
    for core in range(N_CORES):
        b, half = divmod(core, 2)
        in_maps.append(_core_inputs_v1(xq[b, half], kp_pos[b]))

    res = run_bass_kernel_spmd(nc, in_maps, core_ids=list(range(N_CORES)))

    perm = _devrow_to_query(nsub, g)
    inv = np.empty_like(perm)
    inv[perm] = np.arange(perm.size)

    vals = np.empty((B, rays * S, 8), np.float32)
    idx = np.empty((B, rays * S, 8), np.int32)
    half_n = rays // 2 * S
    for core in range(N_CORES):
        b, half = divmod(core, 2)
        ob = res.results[core]["oboth"]
        od = ob.view(np.float32)[:, :8]
        oi = ob.view(np.int32)[:, 8:]
        vals[b, half * half_n : (half + 1) * half_n] = od[inv]
        idx[b, half * half_n : (half + 1) * half_n] = oi[inv]

    return _postprocess(x, vals, idx)


def _postprocess(x, vals, idx):
    """vals: [B, rays*S, 8] ascending distances; idx: keypoint ids (0..511)."""
    rays = T * RO
    vals = vals.reshape(B, rays, S, 8)
    idx = idx.reshape(B, rays, S, 8)

    valid_nb = vals < R
    offset = (NKP * np.arange(B, dtype=np.int32)).reshape(B, 1, 1, 1)
    nb_idx = np.where(valid_nb, idx + offset, -1).astype(np.int32)

    valid_pts = valid_nb[..., 0:1]  # any() == slot 0 since ascending
    csum = np.cumsum(valid_pts.astype(np.int32), axis=-2)
    valid_pts = np.logical_and(valid_pts, csum <= MAX_SHADING_PTS)

    nb_idx = np.where(valid_pts, nb_idx, -1)
    nb_dist = np.where(np.logical_and(valid_pts, valid_nb), vals, 0.0).astype(
        np.float32
    )
    shading = np.where(valid_pts, x.reshape(B, rays, S, 3), 0.0).astype(np.float32)

    num_valid = valid_pts.sum(axis=-2, keepdims=True)
    mask = np.arange(MAX_SHADING_PTS).reshape(1, 1, -1, 1) < num_valid

    return (
        nb_idx.reshape(B, T, RO, S, K),
        shading.reshape(B, T, RO, S, 3),
        nb_dist.reshape(B, T, RO, S, K),
        mask.reshape(B, T, RO, MAX_SHADING_PTS, 1),
    )


# revision 27
# speedup vs baseline: 1.4802x; 1.0945x over previous
"""Trainium2 Bass kernel for nn_Aggregator (retrieval_knn).

Reference computation: for each of B*T*Ro*S = 524288 query points, find the
8 nearest of 512 keypoints (per batch), threshold at R=0.12, cap at 48 valid
points per ray (64 points), emit (neighbor_idx, shading_pts, neighbor_dist,
mask).

Device part (per core, SPMD over 8 cores; core = (batch, ray-half)):
  - PE matmul computes s[q,m] = 2*q.k - |k|^2  (= a2[q] - d2[q,m]) for a
    128-query subtile against C candidate keypoints.
  - DVE max/max_index extract the top-8 values (= 8 smallest d2) + indices.
  - d2 = a2 - val, clamped, sqrt -> 8 ascending distances per query.
Host part: radius mask, per-ray cumsum cap, -1/0 fills, mask construction
(cheap O(N) numpy, exactly mirroring the reference semantics).
"""

import os
import sys

import numpy as np

sys.path.insert(0, "/opt/trn_rl_repo")

from contextlib import ExitStack

import concourse.bass as bass
import concourse.tile as tile
from concourse import bacc, mybir
from concourse.bass_utils import run_bass_kernel_spmd

# Problem constants
B, T, RO, S, _D = 4, 2, 1024, 64, 3
NKP = 512
K = 8
R = 0.12
MAX_SHADING_PTS = 48

N_CORES = 8
NQ_CORE = (B * T * RO * S) // N_CORES  # 65536 queries per core

F32 = mybir.dt.float32
F32R = mybir.dt.float32r
U32 = mybir.dt.uint32


def build_program(nsub, c, nrhs, g, reps=1, bufs=(2, 2, 8, 6, 3)):
    """Build the Bass program.

    nsub: number of 128-query subtiles per core
    c:    candidate keypoints per subtile
    nrhs: number of rhs matrices (1 = shared by all subtiles, else nsub)
    g:    subtiles per output-staging group (nsub % g == 0)
    reps: repeat the whole computation (timing only)

    Outputs one merged tensor oboth [n, 16] uint32: per query row,
    cols 0:8 = f32 bits of the 8 ascending distances, cols 8:16 = the
    raw candidate indices (uint32).
    """
    assert nsub % g == 0
    lb, rb, pb, db, wb = bufs
    n = nsub * 128
    nc = bacc.Bacc("TRN2", target_bir_lowering=False)
    lhs = nc.declare_dram_parameter("lhs", [4, n], F32, isOutput=False)
    rhs = nc.declare_dram_parameter("rhs", [nrhs, 4, c], F32, isOutput=False)
    a2d = nc.declare_dram_parameter("a2d", [128, nsub], F32, isOutput=False)
    oboth = nc.declare_dram_parameter("oboth", [n, 16], U32, isOutput=True)

    with tile.TileContext(nc) as tc, ExitStack() as ctx:
        if reps > 1:
            ctx.enter_context(tc.For_i(0, reps, 1))
        lpool = ctx.enter_context(tc.tile_pool(name="lhs", bufs=lb))
        rpool = ctx.enter_context(tc.tile_pool(name="rhs", bufs=rb))
        apool = ctx.enter_context(tc.tile_pool(name="a2", bufs=1))
        ppool = ctx.enter_context(tc.tile_pool(name="psum", bufs=pb, space="PSUM"))
        dpool = ctx.enter_context(tc.tile_pool(name="d2", bufs=db))
        wpool = ctx.enter_context(tc.tile_pool(name="wide", bufs=wb))

        a2all = apool.tile([128, nsub], F32)
        nc.sync.dma_start(out=a2all[:], in_=a2d[:, :])
        rall = None
        if nrhs == 1:
            rall = rpool.tile([4, c], F32)
            nc.sync.dma_start(out=rall[:], in_=rhs[0])

        skip_post = os.environ.get("SKIP_POST", "")

        def make_post(m, wv, stg, stg3):
            # d2 = a2 - val (clamped to >= 1e-12), dist = sqrt(d2).
            # Emitted split into the NEXT macro's subtile loop so no
            # engine's sequencer blocks at a macro boundary waiting for
            # the whole previous macro to finish.
            def pool_part():
                if skip_post == "all":
                    return
                _we = nc.gpsimd
                a2x = wpool.tile([128, g * 8], F32)
                a2x3 = a2x[:].rearrange("p (g e) -> p g e", e=8)
                for e in range(8):
                    _we.tensor_copy(
                        a2x3[:, :, e], a2all[:, m * g : (m + 1) * g]
                    )
                # stg dist cols = val - a2 = -(d2); min -1e-12 == -max(d2,eps)
                sd = stg3[:, :, 0:8]
                _we.tensor_sub(
                    sd, wv[:].rearrange("p (g e) -> p g e", e=8), a2x3
                )
                _we.tensor_scalar_min(sd, sd, -1e-12)

            def act_part():
                if skip_post == "all":
                    return
                sd = stg3[:, :, 0:8]
                nc.scalar.activation(
                    sd, sd, func=mybir.ActivationFunctionType.Sqrt, scale=-1.0
                )

            def dma_part():
                if skip_post in ("all", "dma"):
                    # still need an output write so tiles release; write stg raw
                    pass
                orows = oboth[m * g * 128 : (m + 1) * g * 128].rearrange(
                    "(p g) e -> p (g e)", p=128
                )
                _ob_eng = nc.sync
                _ob_eng.dma_start(out=orows, in_=stg[:].bitcast(U32))

            return pool_part, act_part, dma_part

        post = None
        for m in range(nsub // g):
            wv = wpool.tile([128, g * 8], F32)
            stg = wpool.tile([128, g * 16], F32)
            stg3 = stg[:].rearrange("p (g e) -> p g e", e=16)
            lc = lpool.tile([4, g * 128], F32)
            nc.sync.dma_start(
                out=lc[:], in_=lhs[:, m * g * 128 : (m + 1) * g * 128]
            )
            rc = None
            if nrhs > 1:
                rc = rpool.tile([4, g * c], F32)
                _rc_eng = nc.gpsimd  # SWDGE: keeps the SP sequencer free
                _rc_eng.dma_start(
                    out=rc[:].rearrange("f (s c) -> f s c", s=g),
                    in_=rhs[m * g : (m + 1) * g].rearrange("s f c -> f s c"),
                )
            p0 = g // 4
            p1 = max(p0 + 1, 2 * g // 3)
            p2 = min(max(p1 + 1, g - 2), g - 1)
            for j in range(g):
                if post is not None:
                    if j == p0:
                        post[0]()
                    elif j == p1:
                        post[1]()
                    elif j == p2:
                        post[2]()
                lslice = lc[:, j * 128 : (j + 1) * 128]
                rslice = rall[:] if nrhs == 1 else rc[:, j * c : (j + 1) * c]
                psum = ppool.tile([128, c], F32)
                nc.tensor.matmul(
                    psum[:], lhsT=lslice, rhs=rslice, start=True, stop=True
                )
                mode = os.environ.get("CORE_MODE", "")
                if mode == "nodve":
                    d2t = dpool.tile([128, c], F32)
                    nc.scalar.copy(out=d2t[:], in_=psum[:])
                    nc.vector.tensor_copy(wv[:, j * 8 : (j + 1) * 8], d2t[:, 0:8])
                    nc.vector.tensor_copy(
                        stg3[:, j, 8:16].bitcast(U32), d2t[:, 0:8].bitcast(U32)
                    )
                elif mode == "nocopy":
                    nc.vector.max(wv[:, j * 8 : (j + 1) * 8], psum[:])
                    nc.vector.max_index(
                        stg3[:, j, 8:16].bitcast(U32),
                        wv[:, j * 8 : (j + 1) * 8],
                        psum[:],
                    )
                else:
                    d2t = dpool.tile([128, c], F32)
                    nc.scalar.copy(out=d2t[:], in_=psum[:])
                    nc.vector.max(wv[:, j * 8 : (j + 1) * 8], d2t[:])
                    nc.vector.max_index(
                        stg3[:, j, 8:16].bitcast(U32),
                        wv[:, j * 8 : (j + 1) * 8],
                        d2t[:],
                    )
            post = make_post(m, wv, stg, stg3)
        for part in post:
            part()
    nc.compile()
    return nc


# ---------------------------------------------------------------------------
# Host-side orchestration
# ---------------------------------------------------------------------------

_PROG_CACHE = {}


def _get_program(nsub, c, nrhs, g):
    key = (nsub, c, nrhs, g)
    if key not in _PROG_CACHE:
        _PROG_CACHE[key] = build_program(nsub, c, nrhs, g)
    return _PROG_CACHE[key]


def _core_inputs_v1(q, kp):
    """Simple dense config: one rhs of all 512 keypoints, identity order.

    q:  [65536, 3] float32 queries of this core
    kp: [512, 3] float32 keypoints of this core's batch
    Returns in_map dict. Device row r maps to query  (s*128 + p)  with
    s = (r // (128*g))*g + r % g,  p = (r // g) % 128.
    """
    nsub, c, g = NQ_CORE // 128, NKP, 64
    lhs = np.empty((4, NQ_CORE), np.float32)
    lhs[:3] = q.T
    lhs[3] = 1.0
    b2 = (kp[:, 0] * kp[:, 0] + kp[:, 1] * kp[:, 1]) + kp[:, 2] * kp[:, 2]
    rhs = np.empty((1, 4, c), np.float32)
    rhs[0, :3] = 2.0 * kp.T
    rhs[0, 3] = -b2
    a2 = (q[:, 0] * q[:, 0] + q[:, 1] * q[:, 1]) + q[:, 2] * q[:, 2]
    a2d = np.ascontiguousarray(a2.reshape(nsub, 128).T)
    return {"lhs": lhs, "rhs": rhs, "a2d": a2d}


def _devrow_to_query(nsub, g):
    """origq[r] for device output row r (v1 ordering)."""
    r = np.arange(nsub * 128)
    m = r // (128 * g)
    j = r % g
    p = (r // g) % 128
    return (m * g + j) * 128 + p


def _devrow_to_slot(nsub, g):
    """slot (= s*128 + p) for device output row r."""
    r = np.arange(nsub * 128)
    m = r // (128 * g)
    j = r % g
    p = (r // g) % 128
    return (m * g + j) * 128 + p


# --- v2: spatial-cell candidate pruning ------------------------------------

D_GRID = 4
C_CAND = 72
G_V2 = 32


def _cell_cand_ids(kp, d):
    """Candidate keypoint id lists per cell (within R+1e-3 of the cell box)."""
    kp64 = kp.astype(np.float64)
    side = 1.0 / d
    thr = (R + 1e-3) ** 2
    out = []
    for i in range(d):
        for j in range(d):
            for l in range(d):
                lo = np.array([i, j, l]) * side
                dd = np.maximum(np.maximum(lo - kp64, 0), kp64 - (lo + side))
                out.append(np.nonzero((dd * dd).sum(1) < thr)[0])
    return out


def _cell_candidates(kp, cand_ids, c):
    """Per-cell rhs [d^3+1, 4, c] and candidate id map [d^3+1, c].

    Last row = dummy (all padded) for pad subtiles. Pad columns get
    [0,0,0,-1e30] so their score 2ab-b2 = -1e30 never wins.
    """
    ncell = len(cand_ids)
    rhs = np.zeros((ncell + 1, 4, c), np.float32)
    rhs[:, 3, :] = -1e30
    cmap = np.zeros((ncell + 1, c), np.int32)
    b2 = (kp[:, 0] * kp[:, 0] + kp[:, 1] * kp[:, 1]) + kp[:, 2] * kp[:, 2]
    for cc, ids in enumerate(cand_ids):
        n = len(ids)
        assert n <= c, f"cell {cc}: {n} candidates > C={c}"
        rhs[cc, 0, :n] = 2.0 * kp[ids, 0]
        rhs[cc, 1, :n] = 2.0 * kp[ids, 1]
        rhs[cc, 2, :n] = 2.0 * kp[ids, 2]
        rhs[cc, 3, :n] = -b2[ids]
        cmap[cc, :n] = ids
    return rhs, cmap


def _assign_subtiles(q, d):
    """Sort queries into cells; chunk each cell into 128-query subtiles.

    Returns (perm2 [nsub_used*128] orig-query index with -1 padding,
             sub_cell [nsub_used] cell id per subtile).
    """
    nq = q.shape[0]
    cid = np.clip((q * d).astype(np.int32), 0, d - 1)
    cell = (cid[:, 0] * d + cid[:, 1]) * d + cid[:, 2]
    order = np.argsort(cell, kind="stable")
    counts = np.bincount(cell, minlength=d**3)
    perm2 = []
    sub_cell = []
    start = 0
    for cc in range(d**3):
        n = int(counts[cc])
        qs = order[start : start + n]
        start += n
        for o in range(0, n, 128):
            chunk = qs[o : o + 128]
            if len(chunk) < 128:
                chunk = np.concatenate(
                    [chunk, np.full(128 - len(chunk), -1, np.int64)]
                )
            perm2.append(chunk)
            sub_cell.append(cc)
    return np.concatenate(perm2), np.asarray(sub_cell, np.int64)


def _core_inputs_v2(q, cell_rhs, cell_cmap, nsub, c, g):
    """Spatial config inputs + mappings for one core."""
    perm2, sub_cell = _assign_subtiles(q, D_GRID)
    nsub_used = len(sub_cell)
    assert nsub_used <= nsub, f"{nsub_used} subtiles > program NSUB={nsub}"
    perm2 = np.concatenate(
        [perm2, np.full((nsub - nsub_used) * 128, -1, np.int64)]
    )
    sub_cell = np.concatenate(
        [sub_cell, np.full(nsub - nsub_used, D_GRID**3, np.int64)]
    )

    rhs = cell_rhs[sub_cell]  # [nsub, 4, c]
    cmap = cell_cmap[sub_cell]  # [nsub, c]

    qsafe = np.where(perm2 >= 0, perm2, 0)
    qc = q[qsafe]  # [nsub*128, 3] slot-ordered coords
    lhs = np.empty((4, nsub * 128), np.float32)
    lhs[:3] = qc.T
    lhs[3] = 1.0
    a2 = (qc[:, 0] * qc[:, 0] + qc[:, 1] * qc[:, 1]) + qc[:, 2] * qc[:, 2]
    a2d = np.ascontiguousarray(a2.reshape(nsub, 128).T)
    return {"lhs": lhs, "rhs": rhs, "a2d": a2d}, perm2, cmap


def _prep_v2(x, kp_pos):
    rays = T * RO
    c, g = C_CAND, G_V2
    xq = x.reshape(B, 2, rays // 2 * S, 3)
    nsub_needed = 0
    for core in range(N_CORES):
        b, half = divmod(core, 2)
        perm2, sub_cell = _assign_subtiles(xq[b, half], D_GRID)
        nsub_needed = max(nsub_needed, len(sub_cell))
    nsub = ((nsub_needed + g - 1) // g) * g

    batch_cands = [_cell_cand_ids(kp_pos[b], D_GRID) for b in range(B)]
    c_needed = max(max(len(ids) for ids in bc) for bc in batch_cands)
    c = max(c, -((-c_needed) // 4) * 4)
    batch_rhs = [
        _cell_candidates(kp_pos[b], batch_cands[b], c) for b in range(B)
    ]

    in_maps, perms, cmaps = [], [], []
    for core in range(N_CORES):
        b, half = divmod(core, 2)
        cell_rhs, cell_cmap = batch_rhs[b]
        im, perm2, cmap = _core_inputs_v2(
            xq[b, half], cell_rhs, cell_cmap, nsub, c, g
        )
        in_maps.append(im)
        perms.append(perm2)
        cmaps.append(cmap)
    return nsub, c, g, in_maps, perms, cmaps


def kernel_v2(x, kp_pos):
    x = np.asarray(x, dtype=np.float32)
    kp_pos = np.asarray(kp_pos, dtype=np.float32)
    rays = T * RO
    nsub, c, g, in_maps, perms, cmaps = _prep_v2(x, kp_pos)

    nc = _get_program(nsub, c, nsub, g)
    res = run_bass_kernel_spmd(nc, in_maps, core_ids=list(range(N_CORES)))

    slot_of_devrow = _devrow_to_slot(nsub, g)
    devrow_of_slot = np.empty_like(slot_of_devrow)
    devrow_of_slot[slot_of_devrow] = np.arange(slot_of_devrow.size)

    vals = np.empty((B, rays * S, 8), np.float32)
    idx = np.empty((B, rays * S, 8), np.int32)
    half_n = rays // 2 * S
    s_of_slot = np.arange(nsub * 128) // 128
    for core in range(N_CORES):
        b, half = divmod(core, 2)
        ob = res.results[core]["oboth"]
        od = ob.view(np.float32)[:, :8][devrow_of_slot]  # slot order
        oi = ob.view(np.int32)[:, 8:][devrow_of_slot]
        kpid = cmaps[core][s_of_slot[:, None], oi]  # [n, 8]
        perm2 = perms[core]
        valid = perm2 >= 0
        dst = vals[b, half * half_n : (half + 1) * half_n]
        dsti = idx[b, half * half_n : (half + 1) * half_n]
        dst[perm2[valid]] = od[valid]
        dsti[perm2[valid]] = kpid[valid]

    return _postprocess(x, vals, idx)


def kernel(x, kp_pos):
    x = np.asarray(x, dtype=np.float32)
    kp_pos = np.asarray(kp_pos, dtype=np.float32)
    rays = T * RO

    nsub, c, nrhs, g = NQ_CORE // 128, NKP, 1, 64
    nc = _get_program(nsub, c, nrhs, g)

    xq = x.reshape(B, 2, rays // 2 * S, 3)
    in_maps = []
    for core in range(N_CORES):
        b, half = divmod(core, 2)
        in_maps.append(_core_inputs_v1(xq[b, half], kp_pos[b]))

    res = run_bass_kernel_spmd(nc, in_maps, core_ids=list(range(N_CORES)))

    perm = _devrow_to_query(nsub, g)
    inv = np.empty_like(perm)
    inv[perm] = np.arange(perm.size)

    vals = np.empty((B, rays * S, 8), np.float32)
    idx = np.empty((B, rays * S, 8), np.int32)
    half_n = rays // 2 * S
    for core in range(N_CORES):
        b, half = divmod(core, 2)
        ob = res.results[core]["oboth"]
        od = ob.view(np.float32)[:, :8]
        oi = ob.view(np.int32)[:, 8:]
        vals[b, half * half_n : (half + 1) * half_n] = od[inv]
        idx[b, half * half_n : (half + 1) * half_n] = oi[inv]

    return _postprocess(x, vals, idx)


def _postprocess(x, vals, idx):
    """vals: [B, rays*S, 8] ascending distances; idx: keypoint ids (0..511)."""
    rays = T * RO
    vals = vals.reshape(B, rays, S, 8)
    idx = idx.reshape(B, rays, S, 8)

    valid_nb = vals < R
    offset = (NKP * np.arange(B, dtype=np.int32)).reshape(B, 1, 1, 1)
    nb_idx = np.where(valid_nb, idx + offset, -1).astype(np.int32)

    valid_pts = valid_nb[..., 0:1]  # any() == slot 0 since ascending
    csum = np.cumsum(valid_pts.astype(np.int32), axis=-2)
    valid_pts = np.logical_and(valid_pts, csum <= MAX_SHADING_PTS)

    nb_idx = np.where(valid_pts, nb_idx, -1)
    nb_dist = np.where(np.logical_and(valid_pts, valid_nb), vals, 0.0).astype(
        np.float32
    )
    shading = np.where(valid_pts, x.reshape(B, rays, S, 3), 0.0).astype(np.float32)

    num_valid = valid_pts.sum(axis=-2, keepdims=True)
    mask = np.arange(MAX_SHADING_PTS).reshape(1, 1, -1, 1) < num_valid

    return (
        nb_idx.reshape(B, T, RO, S, K),
        shading.reshape(B, T, RO, S, 3),
        nb_dist.reshape(B, T, RO, S, K),
        mask.reshape(B, T, RO, MAX_SHADING_PTS, 1),
    )
